# revision 1
# baseline (speedup 1.0000x reference)
"""Trainium2 Bass kernel for nn_AttentionLayer (additive/Luong-style pooling attention).

Reference computation (per node n of N=2048):
    score_T = tanh(W^T @ hs[n] + b)        # (H=512, S=256), hs[n] is (H, S)
    align   = c^T @ score_T                # (S,)
    attn    = softmax(align)               # (S,)
    out[n]  = hs[n] @ attn                 # (H,)

Sharding: data-parallel over nodes, 256 nodes per core across 8 cores.

Per-core dataflow (all on-chip, hs read from HBM exactly once):
  - score matmul on PE in float32r (1 cyc/row), W stationary, hs moving,
    processing node PAIRS (free dim 512) into PSUM
  - tanh+bias fused on ScalarE (per-partition bias, PSUM->SBUF)
  - alignment as M=1 PE matvecs (lhsT=c chunk) accumulating in PSUM rows
  - exp on ScalarE directly from the PSUM row, with accum_out collecting
    softmax denominators Z into a (1,128) row per 128-node block
  - attn row broadcast to 128 partitions via GPSIMD partition_broadcast
  - context = fused multiply+reduce (tensor_tensor_reduce) on VectorE:
    hs chunk (128,256) * attn_bcast -> per-partition sums, written as
    columns of a (128, 4, 128) block accumulator
  - per 128-node block: PE-transpose context columns -> (nodes, H) layout,
    PE-transpose Z row -> column, reciprocal, and one fused
    tensor_scalar_mul (1/Z normalize + PSUM->SBUF) before the output DMA.

Softmax is computed without max-subtraction: |align| <= sum|c| < 28, so
exp is safely within fp32 range by construction.
"""
import os
import sys
import numpy as np

for _p in ("/opt/trn_rl_repo", "/root/.axon_site/_ro/trn_rl_repo"):
    if os.path.isdir(_p) and _p not in sys.path:
        sys.path.insert(0, _p)

N_FULL, H, S = 2048, 512, 256
N_CORES = 8
N_LOC = N_FULL // N_CORES  # 256
P = 128
KC = H // P  # 4 k chunks (input feature dim of W)
MC = H // P  # 4 m chunks (output feature dim of W)


def build_nc(n_loc=N_LOC, block=128):
    import concourse.bass as bass
    import concourse.tile as tile
    from concourse import mybir, bacc, library_config
    from concourse.masks import make_identity
    from contextlib import ExitStack

    f32 = mybir.dt.float32
    f32r = mybir.dt.float32r

    assert n_loc % 2 == 0 and n_loc % block == 0
    npairs = n_loc // 2
    pairs_per_block = block // 2

    nc = bacc.Bacc("TRN2")
    hs_d = nc.declare_dram_parameter("hs", [n_loc, H, S], f32r, isOutput=False)
    w_d = nc.declare_dram_parameter("w", [H, H], f32r, isOutput=False)
    b_d = nc.declare_dram_parameter("b", [H, 1], f32, isOutput=False)
    c_d = nc.declare_dram_parameter("c", [H, 1], f32r, isOutput=False)
    out_d = nc.declare_dram_parameter("out", [n_loc, H], f32, isOutput=True)

    with tile.TileContext(nc) as tc, ExitStack() as ctx:
        consts = ctx.enter_context(tc.tile_pool(name="consts", bufs=1))
        hspool = ctx.enter_context(tc.tile_pool(name="hs", bufs=8))
        scorepool = ctx.enter_context(tc.tile_pool(name="score", bufs=3))
        attnpool = ctx.enter_context(tc.tile_pool(name="attn", bufs=4))
        bcastpool = ctx.enter_context(tc.tile_pool(name="bcast", bufs=3))
        blockpool = ctx.enter_context(tc.tile_pool(name="blk", bufs=2))
        outpool = ctx.enter_context(tc.tile_pool(name="outsb", bufs=2))
        miscpool = ctx.enter_context(tc.tile_pool(name="misc", bufs=2))

        ps_score = ctx.enter_context(tc.tile_pool(name="ps_score", bufs=4, space="PSUM"))
        ps_align = ctx.enter_context(tc.tile_pool(name="ps_align", bufs=2, space="PSUM"))
        ps_zcol = ctx.enter_context(tc.tile_pool(name="ps_zcol", bufs=1, space="PSUM"))
        ps_out = ctx.enter_context(tc.tile_pool(name="ps_out", bufs=1, space="PSUM"))

        nc.gpsimd.load_library(library_config.attn)

        def load_pair(q):
            # ---- load hs pair from HBM: (2, H, S) -> (p, n2, hc, s) ----
            t = hspool.tile([P, 2, KC, S], f32r, tag="hspair")
            nc.sync.dma_start(
                out=t,
                in_=hs_d[2 * q : 2 * q + 2, :, :].rearrange(
                    "n2 (hc p) s -> p n2 hc s", p=P
                ),
            )
            return t

        # First pair load is the critical path to the first matmul - issue it
        # before the (larger) W load, and put the consts on the scalar-engine
        # HWDGE ring so they don't queue behind hs pairs on the sync ring.
        pending = [load_pair(0)]

        # ---- constants ----
        # W: (H, H) = (kc*128+p, m) -> (p, kc, m)
        w_sb = consts.tile([P, KC, H], f32r)
        nc.scalar.dma_start(out=w_sb, in_=w_d[:, :].rearrange("(kc p) m -> p kc m", p=P))
        # b, c: (H, 1) -> (p, mc/kc)
        b_sb = consts.tile([P, MC], f32)
        nc.scalar.dma_start(out=b_sb, in_=b_d[:, :].rearrange("(mc p) one -> p (mc one)", p=P))
        c_sb = consts.tile([P, KC], f32r)
        nc.scalar.dma_start(out=c_sb, in_=c_d[:, :].rearrange("(kc p) one -> p (kc one)", p=P))
        ident = consts.tile([P, P], f32)
        make_identity(nc, ident)

        n_blocks = n_loc // block

        for blk in range(n_blocks):
            # per-block accumulators
            zrow = blockpool.tile([1, block], f32, tag="zrow")
            ctx_sb = blockpool.tile([P, MC, block], f32, tag="ctxsb")

            for pq in range(pairs_per_block):
                q = blk * pairs_per_block + pq
                n0 = 2 * q
                hs_pair = pending.pop(0) if pending else load_pair(q)

                score_sb = scorepool.tile([P, MC, 2, S], f32r, tag="scoresb")
                for mc in range(MC):
                    # ---- score matmul: out (128, 2, 256) += W[kc,mc]^T @ hs[kc] ----
                    sc_ps = ps_score.tile([P, 2, S], f32, tag="scps")
                    for kc in range(KC):
                        nc.tensor.matmul(
                            sc_ps[:, :, :],
                            w_sb[:, kc, mc * P : (mc + 1) * P],
                            hs_pair[:, :, kc, :],
                            start=(kc == 0),
                            stop=(kc == KC - 1),
                        )
                    # ---- tanh + per-partition bias, PSUM -> SBUF ----
                    nc.scalar.activation(
                        out=score_sb[:, mc, :, :],
                        in_=sc_ps[:, :, :],
                        func=mybir.ActivationFunctionType.Tanh,
                        bias=b_sb[:, mc : mc + 1],
                        scale=1.0,
                    )

                # ---- alignment for both nodes at once: (1,2,256) += c[mc]^T @ score_sb[mc] ----
                al_ps = ps_align.tile([1, 2, S], f32, tag="alps")
                for mc in range(MC):
                    nc.tensor.matmul(
                        al_ps[0:1, :, :],
                        c_sb[:, mc : mc + 1],
                        score_sb[:, mc, :, :],
                        start=(mc == 0),
                        stop=(mc == MC - 1),
                    )
                for n2 in range(2):
                    n = n0 + n2
                    col = n - blk * block
                    # ---- exp (no max-sub needed; |align| < 28) + Z accum ----
                    attn_row = attnpool.tile([1, S], f32, tag="attnrow")
                    nc.scalar.activation(
                        out=attn_row,
                        in_=al_ps[0:1, n2, :],
                        func=mybir.ActivationFunctionType.Exp,
                        bias=0.0,
                        scale=1.0,
                        accum_out=zrow[0:1, col : col + 1],
                    )
                    # ---- broadcast attn row to 128 partitions ----
                    bcast = bcastpool.tile([P, S], f32, tag="bcast")
                    nc.gpsimd.partition_broadcast(bcast, attn_row[0:1, :], channels=P)
                    # ---- context: per h-chunk fused mult+reduce over s ----
                    # out is a don't-care; write it through a stride-0 AP to a
                    # 1-column dummy to cut DVE write traffic.
                    scratch = miscpool.tile([P, 1], f32, tag="amrscratch")
                    for hc in range(KC):
                        nc.vector.affine_mul_reduce(
                            out=scratch.broadcast_to((P, S)),
                            accum_out=ctx_sb[:, hc, col : col + 1],
                            in0=hs_pair[:, n2, hc, :],
                            in1=bcast,
                            scale=1.0,
                            bias=0.0,
                        )

            # ---- block epilogue: transpose context + Z, normalize, store ----
            zcol_ps = ps_zcol.tile([block, 1], f32, tag="zcol")
            nc.tensor.transpose(zcol_ps, zrow, ident[0:1, 0:1])
            recip = miscpool.tile([block, 1], f32, tag="recip")
            nc.vector.reciprocal(recip, zcol_ps)

            out_ps = ps_out.tile([block, H], f32, tag="outps")
            for hc in range(MC):
                nc.tensor.transpose(
                    out_ps[:, hc * P : (hc + 1) * P], ctx_sb[:, hc, :], ident
                )
            out_sb = outpool.tile([block, H], f32, tag="outsb")
            nc.vector.tensor_scalar_mul(out_sb, out_ps, recip)
            nc.sync.dma_start(
                out=out_d[blk * block : (blk + 1) * block, :], in_=out_sb
            )

    return nc


_CACHE = {}


def _get_nc():
    if "nc" not in _CACHE:
        nc = build_nc()
        nc.finalize()  # Bacc.finalize runs the full pass pipeline (wait
        # splitting, reg alloc, library/ACT table loads, extended-inst codegen)
        _CACHE["nc"] = nc
    return _CACHE["nc"]


def kernel(hidden_states, attention_weights, attention_bias, context_vector):
    from concourse.bass_utils import run_bass_kernel_spmd

    hs = np.ascontiguousarray(np.asarray(hidden_states, dtype=np.float32))
    w = np.ascontiguousarray(np.asarray(attention_weights, dtype=np.float32))
    b = np.ascontiguousarray(np.asarray(attention_bias, dtype=np.float32))
    c = np.ascontiguousarray(np.asarray(context_vector, dtype=np.float32))

    nc = _get_nc()
    in_maps = [
        {
            "hs": hs[core * N_LOC : (core + 1) * N_LOC],
            "w": w,
            "b": b,
            "c": c,
        }
        for core in range(N_CORES)
    ]
    res = run_bass_kernel_spmd(nc, in_maps, list(range(N_CORES)))
    out = np.concatenate([res.results[i]["out"] for i in range(N_CORES)], axis=0)
    return out.astype(np.float32)


if __name__ == "__main__":
    rng = np.random.default_rng(0)
    hs = rng.standard_normal((N_FULL, H, S)).astype(np.float32)
    w = (rng.standard_normal((H, H)) * 0.05).astype(np.float32)
    b = np.zeros((H, 1), np.float32)
    c = (rng.standard_normal((H, 1)) * 0.1).astype(np.float32)
    out = kernel(hs, w, b, c)
    print(out.shape, out.dtype)

